# revision 6
# baseline (speedup 1.0000x reference)
"""Trainium2 Bass kernel for nn_BinLinear (BN -> binarize -> binary GEMM -> scale -> ReLU).

Reference semantics (for full inputs x[B,IN], weight[OUT,IN], gamma/beta[IN], bias[OUT]):
    mu   = mean(x, axis=0);  var = var(x, axis=0)           (batch stats)
    xn   = (x - mu)/sqrt(var+EPS)*gamma + beta
    xb   = sign(xn)
    wc   = clip(w - rowmean(w), -1, 1); scale = sum(|wc|, axis=1)/IN
    wb   = sign(wc)
    y    = relu((xb @ wb.T + bias) * scale)

Distribution: data-parallel over batch across 8 NeuronCores (1024 rows each).
BN batch stats are computed as per-core partial sums [sum(x), sum(x^2)] per
feature and combined with an in-kernel AllReduce; everything else is local.

Per-core layout choices:
  - x is fed pre-transposed   xt[IN, B_shard] so features sit on SBUF partitions:
    batch-stat reduction is a free-axis reduce, and binarized xb tiles are
    directly usable as the moving matmul operand (contraction dim = feature).
  - w is fed naturally [OUT, IN] so rowmean / L1 scale are free-axis ops;
    binarized wb (bf16, exactly +/-1) is transposed on the tensor engine
    (128x128 blocks via identity matmul) into wbT[f, o] for use as the
    stationary matmul operand.
  - Output is produced transposed yt[OUT, B_shard] so the ReLU+scale epilogue
    is a single scalar-engine activation with per-partition (per-out-channel)
    scale/bias; the host transposes back.

All binarized values are exactly representable in bf16 and products accumulate
exactly in fp32 PSUM (integers <= 4096), so the GEMM is bit-exact.
"""

import numpy as np

import concourse.bass as bass
import concourse.mybir as mybir
import concourse.tile as tile
from concourse import bacc
from concourse import bass_utils
from concourse.masks import make_identity

AF = mybir.ActivationFunctionType
ALU = mybir.AluOpType
F32 = mybir.dt.float32
BF16 = mybir.dt.bfloat16

N_CORES = 8
B_FULL, IN, OUT = 8192, 4096, 4096
EPS = 1e-4


def emit_kernel(tc, outs, ins, *, n_cores, b_shard, d_in, d_out, head=4):
    """Emit the per-core kernel body.

    ins:  dict with APs: xt [d_in, b_shard] f32, w [d_out, d_in] f32,
          gamma2/beta2 [128, d_in//128] f32, bias2 [128, d_out//128] f32
          (column t of the 2D [128, T] layouts holds elements t*128..t*128+127)
    outs: dict with AP: yt [d_out, b_shard] f32
    """
    nc = tc.nc
    ft = d_in // 128   # number of feature tiles
    ot = d_out // 128  # number of output-channel tiles
    assert b_shard % 128 == 0
    nbs = min(512, b_shard)      # matmul moving free dim per block
    nb = b_shard // nbs          # batch blocks
    tg = min(8, ft)              # transpose group size (PSUM bank = 8*128 bf16)
    head = min(head, ot)
    b_total = float(n_cores * b_shard)

    xt, w = ins["xt"], ins["w"]
    gamma2, beta2, bias2 = ins["gamma2"], ins["beta2"], ins["bias2"]
    yt = outs["yt"]

    from contextlib import ExitStack
    ctx = ExitStack()
    xpool = ctx.enter_context(tc.tile_pool(name="xpool", bufs=2))
    trash = ctx.enter_context(tc.tile_pool(name="trash", bufs=2))
    xbpool = ctx.enter_context(tc.tile_pool(name="xbpool", bufs=ft))
    wpool = ctx.enter_context(tc.tile_pool(name="wpool", bufs=2))
    wbpool = ctx.enter_context(tc.tile_pool(name="wbpool", bufs=2))
    wtpool = ctx.enter_context(tc.tile_pool(name="wtpool", bufs=head + 2))
    ypool = ctx.enter_context(tc.tile_pool(name="ypool", bufs=2))
    smalls = ctx.enter_context(tc.tile_pool(name="smalls", bufs=1))
    psum_t = ctx.enter_context(tc.tile_pool(name="psum_t", bufs=2, space="PSUM"))
    psum_mm = ctx.enter_context(tc.tile_pool(name="psum_mm", bufs=4, space="PSUM"))
    dram = ctx.enter_context(tc.tile_pool(name="dram", bufs=1, space="DRAM"))

    # ---- constants / small tiles -------------------------------------------
    ident = smalls.tile([128, 128], BF16)
    make_identity(nc, ident)
    sb_gamma = smalls.tile([128, ft], F32)
    sb_beta = smalls.tile([128, ft], F32)
    sb_bias = smalls.tile([128, ot], F32)
    nc.sync.dma_start(out=sb_gamma[:], in_=gamma2)
    nc.sync.dma_start(out=sb_beta[:], in_=beta2)
    nc.sync.dma_start(out=sb_bias[:], in_=bias2)

    stats = smalls.tile([128, 2 * ft], F32)   # local [sum(x) | sum(x^2)]
    g = smalls.tile([128, 2 * ft], F32)       # global after AllReduce
    mu = smalls.tile([128, ft], F32)
    musq = smalls.tile([128, ft], F32)
    var = smalls.tile([128, ft], F32)
    inv = smalls.tile([128, ft], F32)
    sc = smalls.tile([128, ft], F32)          # inv * gamma
    bi = smalls.tile([128, ft], F32)          # beta - mu * sc
    rowsum = smalls.tile([128, ot], F32)
    negm = smalls.tile([128, ot], F32)
    ssum = smalls.tile([128, ot], F32)
    scale2 = smalls.tile([128, ot], F32)
    bs2 = smalls.tile([128, ot], F32)

    # ---- phase X-A: local batch stats --------------------------------------
    for t in range(ft):
        xtile = xpool.tile([128, b_shard], F32, tag="xt")
        nc.sync.dma_start(out=xtile[:], in_=xt[t * 128:(t + 1) * 128, :])
        nc.vector.tensor_reduce(
            out=stats[:, t:t + 1], in_=xtile[:], axis=mybir.AxisListType.X,
            op=ALU.add,
        )
        sq = trash.tile([128, b_shard], F32, tag="sq")
        nc.scalar.activation(
            out=sq[:], in_=xtile[:], func=AF.Square,
            accum_out=stats[:, ft + t:ft + t + 1],
        )

    # ---- W head tiles (independent of the collective) ----------------------
    wbts = [None] * ot

    def process_w(t):
        wt = wpool.tile([128, d_in], F32, tag="w")
        nc.sync.dma_start(out=wt[:], in_=w[t * 128:(t + 1) * 128, :])
        wb = wbpool.tile([128, d_in], BF16, tag="wb")
        # rowmean via scalar-engine copy with free-axis accumulate
        nc.scalar.activation(
            out=wb[:], in_=wt[:], func=AF.Copy,
            accum_out=rowsum[:, t:t + 1],
        )
        nc.vector.tensor_scalar_mul(negm[:, t:t + 1], rowsum[:, t:t + 1], -1.0 / d_in)
        # wb = sign(w - rowmean)  (bf16, exactly +/-1)
        nc.scalar.activation(
            out=wb[:], in_=wt[:], func=AF.Sign, bias=negm[:, t:t + 1], scale=1.0,
        )
        # ssum = sum(|w - rowmean|) = sum((w - rowmean) * wb), in-place into wt
        nc.vector.scalar_tensor_tensor(
            out=wt[:], in0=wt[:], scalar=negm[:, t:t + 1], in1=wb[:],
            op0=ALU.add, op1=ALU.mult, accum_out=ssum[:, t:t + 1],
        )
        nc.vector.tensor_scalar_mul(scale2[:, t:t + 1], ssum[:, t:t + 1], 1.0 / d_in)
        nc.vector.tensor_tensor(
            out=bs2[:, t:t + 1], in0=sb_bias[:, t:t + 1], in1=scale2[:, t:t + 1],
            op=ALU.mult,
        )
        # transpose wb -> wbT[f, o] in groups of tg 128x128 blocks
        wbt = wtpool.tile([128, ft, 128], BF16, tag="wbt")
        for gidx in range(ft // tg):
            ptile = psum_t.tile([128, tg, 128], BF16, tag="pt")
            for j in range(tg):
                k = gidx * tg + j
                nc.tensor.transpose(
                    ptile[:, j, :], wb[:, k * 128:(k + 1) * 128], ident[:],
                )
            if gidx % 2 == 0:
                nc.vector.tensor_copy(wbt[:, gidx * tg:(gidx + 1) * tg, :], ptile[:])
            else:
                nc.scalar.copy(wbt[:, gidx * tg:(gidx + 1) * tg, :], ptile[:])
        wbts[t] = wbt

    for t in range(head):
        process_w(t)

    # ---- AllReduce of batch stats ------------------------------------------
    if n_cores > 1:
        b_in = dram.tile([128, 2 * ft], F32)
        b_out = dram.tile([128, 2 * ft], F32)
        nc.sync.dma_start(out=b_in[:], in_=stats[:])
        nc.gpsimd.collective_compute(
            "AllReduce", ALU.add,
            replica_groups=[list(range(n_cores))],
            ins=[b_in.opt()], outs=[b_out.opt()],
        )
        nc.sync.dma_start(out=g[:], in_=b_out[:])
        gg = g
    else:
        gg = stats

    # ---- stats math ---------------------------------------------------------
    nc.vector.tensor_scalar_mul(mu[:], gg[:, 0:ft], 1.0 / b_total)
    nc.vector.tensor_tensor(out=musq[:], in0=mu[:], in1=mu[:], op=ALU.mult)
    nc.vector.scalar_tensor_tensor(
        out=var[:], in0=gg[:, ft:2 * ft], scalar=1.0 / b_total, in1=musq[:],
        op0=ALU.mult, op1=ALU.subtract,
    )
    eps_t = smalls.tile([128, 1], F32)
    nc.vector.memset(eps_t[:], EPS)
    nc.scalar.activation(out=var[:], in_=var[:], func=AF.Sqrt, bias=eps_t[:], scale=1.0)
    nc.vector.reciprocal(out=inv[:], in_=var[:])
    nc.vector.tensor_tensor(out=sc[:], in0=inv[:], in1=sb_gamma[:], op=ALU.mult)
    nc.vector.tensor_tensor(out=bi[:], in0=mu[:], in1=sc[:], op=ALU.mult)
    nc.vector.tensor_tensor(out=bi[:], in0=sb_beta[:], in1=bi[:], op=ALU.subtract)

    # ---- phase X-B: reload x, binarize -------------------------------------
    xbs = []
    for t in range(ft):
        xrt = xpool.tile([128, b_shard], F32, tag="xt")
        nc.sync.dma_start(out=xrt[:], in_=xt[t * 128:(t + 1) * 128, :])
        xb = xbpool.tile([128, b_shard], BF16, tag="xb")
        nc.scalar.activation(
            out=xb[:], in_=xrt[:], func=AF.Sign,
            bias=bi[:, t:t + 1], scale=sc[:, t:t + 1],
        )
        xbs.append(xb)

    # ---- W tail + matmul phases --------------------------------------------
    def mm(t):
        wbt = wbts[t]
        psums = [psum_mm.tile([128, nbs], F32, tag="mm", name=f"mm_{t}_{b}")
                 for b in range(nb)]
        for k in range(ft):
            lhs = wbt[:, k, :]
            for b in range(nb):
                nc.tensor.matmul(
                    psums[b], lhs, xbs[k][:, b * nbs:(b + 1) * nbs],
                    start=(k == 0), stop=(k == ft - 1),
                )
        ytile = ypool.tile([128, b_shard], F32, tag="y")
        for b in range(nb):
            nc.scalar.activation(
                out=ytile[:, b * nbs:(b + 1) * nbs], in_=psums[b], func=AF.Relu,
                scale=scale2[:, t:t + 1], bias=bs2[:, t:t + 1],
            )
        nc.sync.dma_start(out=yt[t * 128:(t + 1) * 128, :], in_=ytile[:])
        wbts[t] = None

    for t in range(ot):
        if t + head < ot:
            process_w(t + head)
        mm(t)

    ctx.close()


def _host_prep(x, gamma, beta, weight, bias, n_cores, b_shard, d_in, d_out):
    """Shard + reformat full inputs into per-core input maps."""
    ft, ot = d_in // 128, d_out // 128
    gamma2 = np.ascontiguousarray(np.asarray(gamma, np.float32).reshape(ft, 128).T)
    beta2 = np.ascontiguousarray(np.asarray(beta, np.float32).reshape(ft, 128).T)
    bias2 = np.ascontiguousarray(np.asarray(bias, np.float32).reshape(ot, 128).T)
    w = np.ascontiguousarray(np.asarray(weight, np.float32))
    in_maps = []
    for c in range(n_cores):
        xs = np.asarray(x[c * b_shard:(c + 1) * b_shard], np.float32)
        xtc = np.ascontiguousarray(xs.T)
        in_maps.append({
            "xt": xtc, "w": w,
            "gamma2": gamma2, "beta2": beta2, "bias2": bias2,
        })
    return in_maps


_CACHE = {}


def _build(n_cores, b_shard, d_in, d_out):
    key = (n_cores, b_shard, d_in, d_out)
    if key in _CACHE:
        return _CACHE[key]
    nc = bacc.Bacc("TRN2", target_bir_lowering=False, debug=False,
                   num_devices=n_cores)
    ft, ot = d_in // 128, d_out // 128
    ins = {
        "xt": nc.dram_tensor("xt", [d_in, b_shard], F32, kind="ExternalInput").ap(),
        "w": nc.dram_tensor("w", [d_out, d_in], F32, kind="ExternalInput").ap(),
        "gamma2": nc.dram_tensor("gamma2", [128, ft], F32, kind="ExternalInput").ap(),
        "beta2": nc.dram_tensor("beta2", [128, ft], F32, kind="ExternalInput").ap(),
        "bias2": nc.dram_tensor("bias2", [128, ot], F32, kind="ExternalInput").ap(),
    }
    outs = {
        "yt": nc.dram_tensor("yt", [d_out, b_shard], F32, kind="ExternalOutput").ap(),
    }
    with tile.TileContext(nc) as tc:
        emit_kernel(tc, outs, ins, n_cores=n_cores, b_shard=b_shard,
                    d_in=d_in, d_out=d_out)
    nc.compile()
    _CACHE[key] = nc
    return nc


def kernel(x, gamma, beta, weight, bias):
    b_shard = B_FULL // N_CORES
    nc = _build(N_CORES, b_shard, IN, OUT)
    in_maps = _host_prep(x, gamma, beta, weight, bias, N_CORES, b_shard, IN, OUT)
    res = bass_utils.run_bass_kernel_spmd(
        nc, in_maps, core_ids=list(range(N_CORES)),
    )
    out = np.empty((B_FULL, OUT), np.float32)
    for c in range(N_CORES):
        out[c * b_shard:(c + 1) * b_shard] = res.results[c]["yt"].T
    return out
